# revision 19
# baseline (speedup 1.0000x reference)
"""DeepGCN (4-layer GCN, N=50000 nodes, E=800000 edges, D=128) on 8 Trainium2
NeuronCores via Bass/Tile.

Strategy:
 - Permute nodes into 8 shards x 49 windows of 128 ("slots"), balancing
   in-degree so every (core, slot) has a similar edge count.
 - Each core owns the destination rows of its shard. spmm uses the identity
   A @ (x W) = (A x) W: gather source rows of X (replicated in DRAM via
   AllGather each layer, fp16) with SWDGE dma_gather; the one-hot-times-val
   matrices M per 128-edge chunk are precomputed on HOST (fp16) and streamed
   from DRAM; zT = G^T M accumulates on the PE into PSUM per slot. Then
   h = z @ W_i, PairNorm (global stats via a tiny AllReduce), ReLU, residual,
   and an AllGather of the new shard into the next layer's X table.
 - BatchNorm + fc_in are folded into a single GEMM with on-chip folded
   weights; fc_out is applied per slot at the end.

The int16 gather-index limit (32767) forces a lo/hi split of the X table.
"""

import sys

sys.path.insert(0, "/opt/trn_rl_repo")

import numpy as np

import concourse.bacc as bacc
import concourse.mybir as mybir
import concourse.tile as tile
from concourse.bass_utils import run_bass_kernel_spmd
from concourse.library_config import mlp
from concourse.masks import make_identity

P = 128
NCORES = 8
N = 50000
D = 128
C = 40
L = 4
SLOTS = 49
SLOTS_A = 24                    # X-table A holds slots [0, SLOTS_A) of every core
NS_A = SLOTS_A * P              # 3072 rows per core in table A
NS_B = (SLOTS - SLOTS_A) * P    # 3200 rows per core in table B
MAXCH = 8  # max chunks (128 idxs each) per dma_gather call
EPS_BN = 1e-5
EPS_PN = 1e-6

F32 = mybir.dt.float32
F16 = mybir.dt.float16
I16 = mybir.dt.int16
I32 = mybir.dt.int32

TRACE = False
LAST_EXEC_NS = None

_nc_cache = {}


# ------------------------------------------------------------------ host prep

def _positions(edge_row):
    """Assign nodes to (core, slot, offset) balancing in-degree.

    Returns pos[node] -> global permuted position, and pos2node[pos] -> node
    (-1 for padding positions)."""
    NS = SLOTS * P
    deg = np.bincount(edge_row, minlength=N)
    order = np.argsort(-deg, kind="stable")
    r = np.arange(N)
    rnd, pc = r // NCORES, r % NCORES
    core_of_rank = np.where(rnd % 2 == 0, pc, NCORES - 1 - pc)

    pos = np.empty(N, np.int64)
    for c in range(NCORES):
        nodes_c = order[core_of_rank == c]
        m = len(nodes_c)
        rr = np.arange(m)
        rnd2, ps_ = rr // SLOTS, rr % SLOTS
        slot = np.where(rnd2 % 2 == 0, ps_, SLOTS - 1 - ps_)
        off = rnd2
        pos[nodes_c] = c * NS + slot * P + off

    pos2node = np.full(NCORES * NS, -1, np.int64)
    pos2node[pos] = np.arange(N)
    return pos, pos2node


def _preprocess(edge_row, edge_col, edge_val):
    NS = SLOTS * P
    pos, pos2node = _positions(edge_row)
    pd = pos[edge_row]
    ps = pos[edge_col]
    core = pd // NS
    slotg = (pd % NS) // P
    doff = pd % P
    # source side: split into table A (slots < SLOTS_A of each core) and
    # table B (remaining slots); "lo" = A, "hi" = B below.
    cs = ps // NS
    lloc = ps % NS
    hi = (lloc >= NS_A).astype(np.int64)
    gi = np.where(hi == 0, cs * NS_A + lloc, cs * NS_B + (lloc - NS_A))

    key3 = (core * SLOTS + slotg) * 2 + hi
    cnt = np.bincount(key3, minlength=NCORES * SLOTS * 2).reshape(
        NCORES, SLOTS, 2)
    K_LO = np.ceil(cnt[:, :, 0].max(axis=0) / P).astype(int)
    K_HI = np.ceil(cnt[:, :, 1].max(axis=0) / P).astype(int)

    # global chunk columns: slot-major, lo chunks then hi chunks
    base_lo = np.zeros(SLOTS, int)
    base_hi = np.zeros(SLOTS, int)
    ctr = 0
    for s in range(SLOTS):
        base_lo[s] = ctr
        ctr += K_LO[s]
        base_hi[s] = ctr
        ctr += K_HI[s]
    TOT = ctr
    # index stream chunk bases (lo and hi streams are packed separately, in
    # the same group/slot order)
    sb_lo = np.concatenate([[0], np.cumsum(K_LO)[:-1]])
    sb_hi = np.concatenate([[0], np.cumsum(K_HI)[:-1]])
    KLT, KHT = int(K_LO.sum()), int(K_HI.sum())

    per_core = []
    for c in range(NCORES):
        sel = np.flatnonzero(core == c)
        k = slotg[sel] * 2 + hi[sel]
        si = np.argsort(k, kind="stable")
        es = sel[si]
        ks = k[si]
        m = len(es)
        change = np.r_[True, np.diff(ks) != 0]
        segstart = np.maximum.accumulate(np.where(change, np.arange(m), 0))
        rank = np.arange(m) - segstart

        # host-built one-hot-times-val M tiles: [P(edge), TOT*P] fp16,
        # chunk j occupies cols [j*P, (j+1)*P); M[e, j*P + d] = val_e
        mt = np.zeros((P, max(TOT, 1) * P), np.float16)
        idx_lo_flat = np.zeros(max(KLT, 1) * P, np.int16)
        idx_hi_flat = np.zeros(max(KHT, 1) * P, np.int16)

        for is_hi, base, sbase, flat in (
            (0, base_lo, sb_lo, idx_lo_flat),
            (1, base_hi, sb_hi, idx_hi_flat),
        ):
            msk = hi[es] == is_hi
            ee = es[msk]
            rk = rank[msk]
            sl = slotg[ee]
            gch = base[sl] + rk // P
            mt[rk % P, gch * P + doff[ee]] = edge_val[ee]
            flat[sbase[sl] * P + rk] = gi[ee]

        def wrap(flat, kt):
            a = flat.reshape(kt * 8, 16).T  # [16, cols]
            return np.ascontiguousarray(np.tile(a, (8, 1)))

        per_core.append(dict(
            mtiles=mt,
            idx_lo=wrap(idx_lo_flat, max(KLT, 1)),
            idx_hi=wrap(idx_hi_flat, max(KHT, 1)),
        ))

    sched = (tuple(int(x) for x in K_LO), tuple(int(x) for x in K_HI))
    meta = dict(K_LO=K_LO, K_HI=K_HI, base_lo=base_lo, base_hi=base_hi,
                sb_lo=sb_lo, sb_hi=sb_hi, TOT=TOT, KLT=KLT, KHT=KHT)
    return pos, pos2node, per_core, sched, meta


# ------------------------------------------------------------------ bass build

def _build(meta):
    K_LO, K_HI = meta["K_LO"], meta["K_HI"]
    base_lo = meta["base_lo"]
    sb_lo, sb_hi = meta["sb_lo"], meta["sb_hi"]
    TOT, KLT, KHT = meta["TOT"], meta["KLT"], meta["KHT"]
    NS = SLOTS * P
    NTOT = NCORES * NS
    OP = mybir.AluOpType
    AF = mybir.ActivationFunctionType
    AX = mybir.AxisListType

    nc = bacc.Bacc("TRN2", target_bir_lowering=False, debug=False,
                   num_devices=NCORES, num_swdge_queues=4)

    xt_own = nc.dram_tensor("xt_own", [P, NS], F32, kind="ExternalInput")
    idx_lo = nc.dram_tensor("idx_lo", [P, max(KLT, 1) * 8], I16,
                            kind="ExternalInput")
    idx_hi = nc.dram_tensor("idx_hi", [P, max(KHT, 1) * 8], I16,
                            kind="ExternalInput")
    mtiles = nc.dram_tensor("mtiles", [P, max(TOT, 1) * P], F16,
                            kind="ExternalInput")
    fc_in_w = nc.dram_tensor("fc_in_w", [D, D], F32, kind="ExternalInput")
    fc_in_b = nc.dram_tensor("fc_in_b", [1, D], F32, kind="ExternalInput")
    bn_g = nc.dram_tensor("bn_g", [1, D], F32, kind="ExternalInput")
    bn_b = nc.dram_tensor("bn_b", [1, D], F32, kind="ExternalInput")
    gc_w = nc.dram_tensor("gc_w", [L * D, D], F16, kind="ExternalInput")
    fc_out_w = nc.dram_tensor("fc_out_w", [D, C], F32, kind="ExternalInput")
    fc_out_b = nc.dram_tensor("fc_out_b", [1, C], F32, kind="ExternalInput")
    out = nc.dram_tensor("out", [NS, C], F32, kind="ExternalOutput")

    RG = [list(range(NCORES))]

    with tile.TileContext(nc) as tc:
        nc.gpsimd.load_library(mlp)
        with (
            tc.tile_pool(name="const", bufs=1) as cp,
            tc.tile_pool(name="meta", bufs=1) as mp_,
            tc.tile_pool(name="big", bufs=1) as bp,
            tc.tile_pool(name="gpoolA", bufs=16) as gpa,
            tc.tile_pool(name="gpoolB", bufs=3) as gpb,
            tc.tile_pool(name="mpool", bufs=3) as mpl,
            tc.tile_pool(name="work", bufs=2) as wp,
            tc.tile_pool(name="small", bufs=1) as sp,
            tc.tile_pool(name="dram", bufs=1, space="DRAM") as dp,
        ):
            # ---------------- constants / inputs to SBUF
            ident = cp.tile([P, P], F32)
            make_identity(nc, ident[:])
            ones_col = cp.tile([P, 1], F32)
            nc.vector.memset(ones_col[:], 1.0)
            ones_row = cp.tile([1, P], F32)
            nc.vector.memset(ones_row[:], 1.0)
            eps_bn_t = cp.tile([P, 1], F32)
            nc.vector.memset(eps_bn_t[:], EPS_BN)
            eps_pn_t = cp.tile([1, 1], F32)
            nc.vector.memset(eps_pn_t[:], EPS_PN)

            w1_raw = cp.tile([D, D], F32)
            nc.sync.dma_start(w1_raw[:], fc_in_w[:])
            fcb_s = cp.tile([1, D], F32)
            nc.sync.dma_start(fcb_s[:], fc_in_b[:])
            bn_s = cp.tile([2, D], F32)
            nc.sync.dma_start(bn_s[0:1, :], bn_g[:])
            nc.sync.dma_start(bn_s[1:2, :], bn_b[:])
            gw_s = [cp.tile([D, D], F16, tag=f"gw{i}", name=f"gw{i}")
                    for i in range(L)]
            for i in range(L):
                nc.sync.dma_start(gw_s[i][:], gc_w[i * D:(i + 1) * D, :])
            wo_s = cp.tile([D, C], F32)
            nc.sync.dma_start(wo_s[:], fc_out_w[:])
            bo_s = cp.tile([1, C], F32)
            nc.sync.dma_start(bo_s[:], fc_out_b[:])

            idx_lo_s = mp_.tile([P, max(KLT, 1) * 8], I16)
            nc.sync.dma_start(idx_lo_s[:], idx_lo[:])
            idx_hi_s = mp_.tile([P, max(KHT, 1) * 8], I16)
            nc.sync.dma_start(idx_hi_s[:], idx_hi[:])

            xt_s = bp.tile([P, NS], F32)
            nc.sync.dma_start(xt_s[:], xt_own[:])

            xn = [bp.tile([P, P], F32, tag=f"xn{s}", name=f"xn{s}")
                  for s in range(SLOTS)]
            hs = [bp.tile([P, P], F32, tag=f"h{s}", name=f"hs{s}")
                  for s in range(SLOTS)]

            # DRAM internals (X tables Shared for fast AllGather output;
            # Shared tensors are single-writer, so one table per layer and
            # per half: A = slots [0, SLOTS_A), B = the rest — both halves
            # stay under the int16 gather-index limit)
            X_ta = [dp.tile([NCORES * NS_A, P], F16, addr_space="Shared",
                            tag=f"XA{i}", name=f"XA{i}") for i in range(L)]
            X_tb = [dp.tile([NCORES * NS_B, P], F16, addr_space="Shared",
                            tag=f"XB{i}", name=f"XB{i}") for i in range(L)]
            ag_in = dp.tile([NS, P], F16)
            st_in = dp.tile([P, 2], F32)
            st_out = dp.tile([P, 2], F32)

            # ---------------- phase 0: BN stats + folded fc_in
            with (
                tc.tile_pool(name="p0psum", bufs=1, space="PSUM") as pp0,
                tc.tile_pool(name="p0sb", bufs=1) as sp0,
            ):
                colsum_o = sp0.tile([P, 1], F32)
                sumsq_o = sp0.tile([P, 1], F32)
                scratch = sp0.tile([P, NS], F32)
                nc.vector.tensor_reduce(colsum_o[:], xt_s[:], axis=AX.X,
                                        op=OP.add)
                nc.scalar.activation(scratch[:], xt_s[:], AF.Square,
                                     accum_out=sumsq_o[:])
                st2 = sp0.tile([P, 2], F32)
                nc.vector.tensor_copy(st2[:, 0:1], colsum_o[:])
                nc.vector.tensor_copy(st2[:, 1:2], sumsq_o[:])
                nc.sync.dma_start(st_in[:], st2[:])
                nc.gpsimd.collective_compute(
                    "AllReduce", OP.add, replica_groups=RG,
                    ins=[st_in[:]], outs=[st_out[:]])
                stg = sp0.tile([P, 2], F32)
                nc.sync.dma_start(stg[:], st_out[:])

                mu = sp0.tile([P, 1], F32)
                nc.vector.tensor_scalar_mul(mu[:], stg[:, 0:1], 1.0 / N)
                msq = sp0.tile([P, 1], F32)
                nc.vector.tensor_scalar_mul(msq[:], stg[:, 1:2], 1.0 / N)
                mu2 = sp0.tile([P, 1], F32)
                nc.vector.tensor_tensor(mu2[:], mu[:], mu[:], op=OP.mult)
                var = sp0.tile([P, 1], F32)
                nc.vector.tensor_tensor(var[:], msq[:], mu2[:],
                                        op=OP.subtract)
                sd = sp0.tile([P, 1], F32)
                nc.scalar.activation(sd[:], var[:], AF.Sqrt,
                                     bias=eps_bn_t[:])
                rs = sp0.tile([P, 1], F32)
                nc.vector.reciprocal(rs[:], sd[:])

                bnT_ps = pp0.tile([P, 2], F32, space="PSUM", tag="pp0a")
                nc.tensor.transpose(bnT_ps[:], bn_s[:], ident[:2, :2])
                bnT = sp0.tile([P, 2], F32)
                nc.scalar.copy(bnT[:], bnT_ps[:])
                a_t = sp0.tile([P, 1], F32)
                nc.vector.tensor_tensor(a_t[:], bnT[:, 0:1], rs[:],
                                        op=OP.mult)
                t2 = sp0.tile([P, 1], F32)
                nc.vector.tensor_tensor(t2[:], mu[:], a_t[:], op=OP.mult)
                csh = sp0.tile([P, 1], F32)
                nc.vector.tensor_tensor(csh[:], bnT[:, 1:2], t2[:],
                                        op=OP.subtract)
                W1 = sp0.tile([D, D], F32)
                nc.vector.tensor_scalar_mul(W1[:], w1_raw[:], a_t[:])
                bp_ps = pp0.tile([1, D], F32, space="PSUM", tag="pp0a")
                nc.tensor.matmul(bp_ps[:], lhsT=csh[:], rhs=w1_raw[:],
                                 start=True, stop=True)
                b1 = sp0.tile([1, D], F32)
                nc.scalar.copy(b1[:], bp_ps[:])
                nc.vector.tensor_tensor(b1[:], b1[:], fcb_s[:], op=OP.add)

                with tc.tile_pool(name="p0g", bufs=3, space="PSUM") as ppg:
                    for s in range(SLOTS):
                        g_ps = ppg.tile([P, D], F32, space="PSUM", tag="g0")
                        nc.tensor.matmul(
                            g_ps[:], lhsT=xt_s[:, s * P:(s + 1) * P],
                            rhs=W1[:], start=True, stop=False)
                        nc.tensor.matmul(g_ps[:], lhsT=ones_row[:],
                                         rhs=b1[:], start=False, stop=True)
                        x0 = sp0.tile([P, D], F16, tag="x0")
                        nc.scalar.copy(x0[:], g_ps[:])
                        nc.sync.dma_start(ag_in[s * P:(s + 1) * P, :], x0[:])
                        if s == SLOTS_A - 1:
                            nc.gpsimd.collective_compute(
                                "AllGather", OP.bypass, replica_groups=RG,
                                ins=[ag_in[0:NS_A, :]], outs=[X_ta[0][:]])
                nc.gpsimd.collective_compute(
                    "AllGather", OP.bypass, replica_groups=RG,
                    ins=[ag_in[NS_A:NS, :]], outs=[X_tb[0][:]])

            # ---------------- layers
            for li in range(L):
                XIN_A, XIN_B = X_ta[li], X_tb[li]
                with (
                    tc.tile_pool(name=f"l{li}ps", bufs=1, space="PSUM") as lp,
                    tc.tile_pool(name=f"l{li}st", bufs=1, space="PSUM") as sps,
                ):
                    colsum_ps = sps.tile([P, 1], F32, space="PSUM",
                                         tag="colsum")
                    sumsq_ps = sps.tile([P, 1], F32, space="PSUM",
                                        tag="sumsq")
                    # software pipeline: issue table-A gathers LA slots ahead
                    # of the matmul loop so they fill the AllGather-B window
                    # (GpSimd executes in order; B gathers block on X_tb).
                    LA = 16
                    max_ka = int(K_LO.max())
                    max_kb = int(K_HI.max())
                    qctr = 0
                    gta = {}
                    gtb = {}

                    def issue_a(s):
                        nonlocal qctr
                        ka = int(K_LO[s])
                        gA = gpa.tile([P, max_ka, P], F16, tag="GA",
                                      name=f"gA{li}_{s}")
                        gta[s] = gA
                        for b0 in range(0, ka, MAXCH):
                            kk = min(MAXCH, ka - b0)
                            c0 = int(sb_lo[s]) + b0
                            nc.gpsimd.dma_gather(
                                gA[:, b0:b0 + kk, :], XIN_A[:],
                                idx_lo_s[:, c0 * 8:(c0 + kk) * 8],
                                kk * P, kk * P, P,
                                queue_num=qctr % 4)
                            qctr += 1

                    def issue_b(s):
                        nonlocal qctr
                        kb = int(K_HI[s])
                        gB = gpb.tile([P, max_kb, P], F16, tag="GB",
                                      name=f"gB{li}_{s}")
                        gtb[s] = gB
                        for b0 in range(0, kb, MAXCH):
                            kk = min(MAXCH, kb - b0)
                            c0 = int(sb_hi[s]) + b0
                            nc.gpsimd.dma_gather(
                                gB[:, b0:b0 + kk, :], XIN_B[:],
                                idx_hi_s[:, c0 * 8:(c0 + kk) * 8],
                                kk * P, kk * P, P,
                                queue_num=qctr % 4)
                            qctr += 1

                    for s in range(SLOTS):
                        klo, khi = int(K_LO[s]), int(K_HI[s])
                        ng = klo + khi
                        if s == 0:
                            for t in range(min(LA, SLOTS)):
                                issue_a(t)
                        elif s + LA - 1 < SLOTS:
                            issue_a(s + LA - 1)
                        issue_b(s)
                        # stream this slot's M tiles (contiguous chunk range)
                        Mt = mpl.tile([P, ng, P], F16, tag="M")
                        g0 = int(base_lo[s])
                        nc.sync.dma_start(
                            Mt[:], mtiles[:, g0 * P:(g0 + ng) * P])
                        zT = lp.tile([P, P], F32, space="PSUM",
                                     tag="zT", bufs=2)
                        for j in range(ng):
                            lhs = (gta[s][:, j, :] if j < klo
                                   else gtb[s][:, j - klo, :])
                            nc.tensor.matmul(
                                zT[:], lhsT=lhs, rhs=Mt[:, j, :],
                                start=(j == 0),
                                stop=(j == ng - 1))
                        zs = wp.tile([P, P], F16, tag="zs")
                        nc.scalar.copy(zs[:], zT[:])
                        h_ps = lp.tile([P, P], F32, space="PSUM",
                                       tag="h", bufs=2)
                        nc.tensor.matmul(h_ps[:], lhsT=zs[:],
                                         rhs=gw_s[li][:],
                                         start=True, stop=True)
                        nc.scalar.copy(hs[s][:], h_ps[:])
                        nc.tensor.matmul(
                            colsum_ps[:], lhsT=hs[s][:], rhs=ones_col[:],
                            start=(s == 0), stop=(s == SLOTS - 1))
                        sq = wp.tile([P, P], F32, tag="sq")
                        nc.scalar.square(sq[:], hs[s][:])
                        nc.tensor.matmul(
                            sumsq_ps[:], lhsT=sq[:], rhs=ones_col[:],
                            start=(s == 0), stop=(s == SLOTS - 1))

                    # PairNorm stats -> AllReduce -> scalars
                    st2 = sp.tile([P, 2], F32, tag="st2")
                    nc.scalar.copy(st2[:, 0:1], colsum_ps[:])
                    nc.scalar.copy(st2[:, 1:2], sumsq_ps[:])
                    nc.sync.dma_start(st_in[:], st2[:])
                    nc.gpsimd.collective_compute(
                        "AllReduce", OP.add, replica_groups=RG,
                        ins=[st_in[:]], outs=[st_out[:]])
                    stg = sp.tile([P, 2], F32, tag="stg")
                    nc.sync.dma_start(stg[:], st_out[:])

                    cmean = sp.tile([P, 1], F32, tag="cmean")
                    nc.vector.tensor_scalar_mul(cmean[:], stg[:, 0:1],
                                                1.0 / N)
                    csq = sp.tile([P, 1], F32, tag="csq")
                    nc.vector.tensor_tensor(csq[:], stg[:, 0:1],
                                            stg[:, 0:1], op=OP.mult)
                    nc.vector.tensor_scalar_mul(csq[:], csq[:], 1.0 / N)
                    q = sp.tile([P, 1], F32, tag="q")
                    nc.vector.tensor_tensor(q[:], stg[:, 1:2], csq[:],
                                            op=OP.subtract)
                    tot_ps = lp.tile([1, 1], F32, space="PSUM", tag="h",
                                     bufs=2)
                    nc.tensor.matmul(tot_ps[:], lhsT=q[:], rhs=ones_col[:],
                                     start=True, stop=True)
                    tot_s = sp.tile([1, 1], F32, tag="tot")
                    nc.scalar.copy(tot_s[:], tot_ps[:])
                    rn = sp.tile([1, 1], F32, tag="rn")
                    nc.scalar.activation(rn[:], tot_s[:], AF.Sqrt,
                                         bias=eps_pn_t[:], scale=1.0 / N)
                    sres = sp.tile([1, 1], F32, tag="sres")
                    nc.vector.reciprocal(sres[:], rn[:])
                    sbc_ps = lp.tile([P, 1], F32, space="PSUM", tag="h",
                                     bufs=2)
                    nc.tensor.matmul(sbc_ps[:], lhsT=ones_row[:],
                                     rhs=sres[:], start=True, stop=True)
                    sbc = sp.tile([P, 1], F32, tag="sbc")
                    nc.scalar.copy(sbc[:], sbc_ps[:])
                    cmb_ps = lp.tile([P, P], F32, space="PSUM", tag="zT",
                                     bufs=2)
                    nc.tensor.transpose(cmb_ps[:],
                                        cmean[:].to_broadcast([P, P]),
                                        ident[:])
                    cmb = sp.tile([P, P], F32, tag="cmb")
                    nc.scalar.copy(cmb[:], cmb_ps[:])

                    # pass 2: x_new = relu(s * (h - colmean)) + x_old
                    # (for the last layer, fc_out is fused into this loop)
                    with tc.tile_pool(name=f"fo{li}", bufs=1,
                                      space="PSUM") as fp:
                        for s in range(SLOTS):
                            t = wp.tile([P, P], F32, tag="t")
                            nc.vector.tensor_tensor(t[:], hs[s][:], cmb[:],
                                                    op=OP.subtract)
                            if li == 0:
                                nc.scalar.activation(xn[s][:], t[:], AF.Relu,
                                                     scale=sbc[:])
                            else:
                                r = wp.tile([P, P], F32, tag="r")
                                nc.scalar.activation(r[:], t[:], AF.Relu,
                                                     scale=sbc[:])
                                nc.vector.tensor_tensor(xn[s][:], r[:],
                                                        xn[s][:], op=OP.add)
                            if li < L - 1:
                                x16 = wp.tile([P, P], F16, tag="x16")
                                nc.vector.tensor_copy(x16[:], xn[s][:])
                                nc.sync.dma_start(
                                    ag_in[s * P:(s + 1) * P, :], x16[:])
                                if s == SLOTS_A - 1:
                                    nc.gpsimd.collective_compute(
                                        "AllGather", OP.bypass,
                                        replica_groups=RG,
                                        ins=[ag_in[0:NS_A, :]],
                                        outs=[X_ta[li + 1][:]])
                            else:
                                tp_ps = fp.tile([P, P], F32, space="PSUM",
                                                tag="tp")
                                nc.tensor.transpose(tp_ps[:], xn[s][:],
                                                    ident[:])
                                xt4 = wp.tile([P, P], F32, tag="xt4")
                                nc.scalar.copy(xt4[:], tp_ps[:])
                                o_ps = fp.tile([P, C], F32, space="PSUM",
                                               tag="o")
                                nc.tensor.matmul(o_ps[:], lhsT=xt4[:],
                                                 rhs=wo_s[:],
                                                 start=True, stop=False)
                                nc.tensor.matmul(o_ps[:], lhsT=ones_row[:],
                                                 rhs=bo_s[:],
                                                 start=False, stop=True)
                                o_s = wp.tile([P, C], F32, tag="os")
                                nc.scalar.copy(o_s[:], o_ps[:])
                                nc.sync.dma_start(out[s * P:(s + 1) * P, :],
                                                  o_s[:])
                    if li < L - 1:
                        nc.gpsimd.collective_compute(
                            "AllGather", OP.bypass, replica_groups=RG,
                            ins=[ag_in[NS_A:NS, :]], outs=[X_tb[li + 1][:]])

    nc.compile()
    return nc


# ------------------------------------------------------------------ kernel

def kernel(x, edge_row, edge_col, edge_val, bn_gamma, bn_beta,
           fc_in_w, fc_in_b, gc_w, gc_b, fc_out_w, fc_out_b):
    global LAST_EXEC_NS
    x = np.asarray(x, np.float32)
    edge_row = np.asarray(edge_row).astype(np.int64)
    edge_col = np.asarray(edge_col).astype(np.int64)
    edge_val = np.asarray(edge_val, np.float32)

    NS = SLOTS * P
    pos, pos2node, per_core, sched, meta = _preprocess(
        edge_row, edge_col, edge_val)

    if sched not in _nc_cache:
        _nc_cache[sched] = _build(meta)
    nc = _nc_cache[sched]

    # xT_own per core: columns = permuted positions of the core's shard
    x_pad = np.zeros((NCORES * NS, D), np.float32)
    x_pad[pos] = x
    shared = dict(
        fc_in_w=np.ascontiguousarray(fc_in_w, dtype=np.float32),
        fc_in_b=np.asarray(fc_in_b, np.float32).reshape(1, D),
        bn_g=np.asarray(bn_gamma, np.float32).reshape(1, D),
        bn_b=np.asarray(bn_beta, np.float32).reshape(1, D),
        gc_w=np.ascontiguousarray(
            np.asarray(gc_w, np.float16).reshape(L * D, D)),
        fc_out_w=np.ascontiguousarray(fc_out_w, dtype=np.float32),
        fc_out_b=np.asarray(fc_out_b, np.float32).reshape(1, C),
    )
    in_maps = []
    for c in range(NCORES):
        m = dict(shared)
        m["xt_own"] = np.ascontiguousarray(
            x_pad[c * NS:(c + 1) * NS].T)
        m.update(per_core[c])
        in_maps.append(m)

    res = run_bass_kernel_spmd(nc, in_maps, list(range(NCORES)),
                               trace=TRACE)
    LAST_EXEC_NS = res.exec_time_ns
    globals()["LAST_RES"] = res

    out_full = np.zeros((N, C), np.float32)
    for c in range(NCORES):
        rows = res.results[c]["out"]
        nodes = pos2node[c * NS:(c + 1) * NS]
        v = nodes >= 0
        out_full[nodes[v]] = rows[v]
    return out_full
